# revision 8
# baseline (speedup 1.0000x reference)
"""Multi-head attention (B=2, S=2048, D=1024, H=16, causal, unscaled scores)
on 8 Trainium2 NeuronCores.

Sharding: 2 batches x 4 head-groups (4 heads each). Core c handles batch
c//4, heads 4*(c%4) .. 4*(c%4)+3. Each core computes its group's QKV
projections, causal attention, and a partial output projection
(row-slice of wo); the host sums the 4 partials per batch (the
all-reduce) and adds the bias terms.

Device layout avoids all on-chip transposes:
  - host passes q/k/v transposed ([D, S]) so projections produce
    QHT/KHT = (x@w).T with head-dim on partitions (score-ready)
  - VH is produced in natural [S, D_head] orientation with an extra
    ones column, so the attnV matmul also accumulates the softmax
    denominator (row 64 of U^T)
  - normalization is applied to U^T via reciprocal + a K=1 broadcast
    matmul, and the bias terms bv/bo are folded in exactly on the host
    (C = U/colsum + 1*bv since softmax rows sum to 1).
All matmuls run as float32r (bf16-pair fp32: ~1e-4 rel err, 4x the
throughput of plain fp32).
"""

import numpy as np

D = 1024
S = 2048
NH = 16
DH = 64
B = 2
G = 4            # head-groups = cores per batch
HG = NH // G     # 4 heads per group
GD = HG * DH     # 256 columns per group
KT = D // 128    # 8 k-tiles
MS = S // 512    # 4 m-slices
JT = S // 128    # 16 j-tiles
IST = S // 512   # 4 i-slices

_cached = None


def _build():
    from concourse import bacc
    import concourse.mybir as mybir
    import concourse.tile as tile

    f32 = mybir.dt.float32
    f32r = mybir.dt.float32r
    Act = mybir.ActivationFunctionType
    Alu = mybir.AluOpType

    nc = bacc.Bacc(None, target_bir_lowering=False)
    xq = nc.dram_tensor("xq", [D, S], f32r, kind="ExternalInput")
    xk = nc.dram_tensor("xk", [D, S], f32r, kind="ExternalInput")
    xv = nc.dram_tensor("xv", [D, S], f32r, kind="ExternalInput")
    wqg = nc.dram_tensor("wqg", [D, GD], f32r, kind="ExternalInput")
    wkg = nc.dram_tensor("wkg", [D, GD], f32r, kind="ExternalInput")
    wvg = nc.dram_tensor("wvg", [D, GD], f32r, kind="ExternalInput")
    wog = nc.dram_tensor("wog", [GD, D], f32r, kind="ExternalInput")
    bqg = nc.dram_tensor("bqg", [2, 128, 1], f32, kind="ExternalInput")
    bkg = nc.dram_tensor("bkg", [2, 128, 1], f32, kind="ExternalInput")
    outp = nc.dram_tensor("outp", [S, D], f32, kind="ExternalOutput")

    with tile.TileContext(nc) as tc:
        with (
            tc.tile_pool(name="wpool", bufs=1) as wpool,
            tc.tile_pool(name="xres", bufs=2) as xres,
            tc.tile_pool(name="xstr", bufs=6) as xstr,
            tc.tile_pool(name="big", bufs=1) as big,
            tc.tile_pool(name="ppool", bufs=4) as ppool,
            tc.tile_pool(name="small", bufs=4) as small,
            tc.tile_pool(name="osb", bufs=3) as osb,
            tc.tile_pool(name="ps", bufs=4, space="PSUM") as ps,
            tc.tile_pool(name="psU", bufs=2, space="PSUM") as psU,
            tc.tile_pool(name="psB", bufs=2, space="PSUM") as psB,
        ):
            # ---- resident weights / constants ----
            wq_t = wpool.tile([128, KT, GD], f32r, tag="wq")
            wk_t = wpool.tile([128, KT, GD], f32r, tag="wk")
            wv_t = wpool.tile([128, KT, GD], f32r, tag="wv")
            wo_t = wpool.tile([128, 2, D], f32r, tag="wo")
            bq_t = wpool.tile([128, 2, 1], f32, tag="bq")
            bk_t = wpool.tile([128, 2, 1], f32, tag="bk")
            ones64 = wpool.tile([1, DH], f32r, tag="ones")
            ones_stage = wpool.tile([1, DH], f32, tag="ones_st")
            nc.sync.dma_start(out=wq_t, in_=xq_like_w(wqg))
            nc.sync.dma_start(out=wk_t, in_=xq_like_w(wkg))
            nc.sync.dma_start(out=wv_t, in_=xq_like_w(wvg))
            nc.sync.dma_start(out=wo_t, in_=wog[:].rearrange("(t p) n -> p t n", p=128))
            nc.sync.dma_start(out=bq_t, in_=bqg[:].rearrange("t p o -> p t o"))
            nc.sync.dma_start(out=bk_t, in_=bkg[:].rearrange("t p o -> p t o"))
            nc.vector.memset(ones_stage, 1.0)
            nc.scalar.activation(out=ones64, in_=ones_stage, func=Act.Copy)

            qht = big.tile([128, 2, S], f32r, tag="qht")
            kht = big.tile([128, 2, S], f32r, tag="kht")
            vh = big.tile([128, JT, HG, DH + 1], f32r, tag="vh")
            ct = big.tile([128, 2, S], f32r, tag="ct")
            vh_ones_stage = wpool.tile([128, JT, HG, 1], f32, tag="vh_ones_st")
            nc.vector.memset(vh_ones_stage, 1.0)
            nc.scalar.activation(
                out=vh[:, :, :, DH : DH + 1], in_=vh_ones_stage, func=Act.Copy
            )

            # ---- Q/K projections: QHT[n, m] = (x @ w).T, bias folded in ----
            for src, w_t, b_t, dst in ((xq, wq_t, bq_t, qht), (xk, wk_t, bk_t, kht)):
                for m in range(MS):
                    xts = []
                    for kk in range(KT):
                        xt = xstr.tile([128, 512], f32r, tag="xt")
                        nc.sync.dma_start(
                            out=xt, in_=src[kk * 128 : (kk + 1) * 128, m * 512 : (m + 1) * 512]
                        )
                        xts.append(xt)
                    for n in range(2):
                        psum = ps.tile([128, 512], f32, tag="ps")
                        for kk in range(KT):
                            nc.tensor.matmul(
                                psum,
                                w_t[:, kk, n * 128 : (n + 1) * 128],
                                xts[kk],
                                start=(kk == 0),
                                stop=(kk == KT - 1),
                            )
                        nc.scalar.activation(
                            out=dst[:, n, m * 512 : (m + 1) * 512],
                            in_=psum,
                            func=Act.Identity,
                            bias=b_t[:, n, :],
                        )

            # ---- V projection (natural orientation), xv streamed in 512-col groups ----
            for mg in range(MS):
                xv_t = xres.tile([128, KT, 512], f32r, tag="xv")
                for kk in range(KT):
                    nc.sync.dma_start(
                        out=xv_t[:, kk, :],
                        in_=xv[kk * 128 : (kk + 1) * 128, mg * 512 : (mg + 1) * 512],
                    )
                for jj in range(4):
                    j = mg * 4 + jj
                    psum = ps.tile([128, GD], f32, tag="ps")
                    for kk in range(KT):
                        nc.tensor.matmul(
                            psum,
                            xv_t[:, kk, jj * 128 : (jj + 1) * 128],
                            wv_t[:, kk, :],
                            start=(kk == 0),
                            stop=(kk == KT - 1),
                        )
                    for h in range(HG):
                        nc.vector.tensor_copy(
                            vh[:, j, h, 0:DH], psum[:, h * DH : (h + 1) * DH]
                        )

            # ---- attention + output projection ----
            for IS in range(IST):
                i0 = IS * 512
                n_j = (IS + 1) * 4
                for h in range(HG):
                    pt_hi = 64 * (h % 2)
                    nt = h // 2
                    k_ap = kht[pt_hi : pt_hi + DH, nt, :]
                    q_ap = qht[pt_hi : pt_hi + DH, nt, i0 : i0 + 512]

                    u_psum = psU.tile([128, 512], f32, tag="u")
                    pts = [None] * n_j
                    s_psums = [None] * n_j

                    def emit_scores(J):
                        s_psum = ps.tile([128, 512], f32, tag="ps")
                        nc.tensor.matmul(
                            s_psum,
                            k_ap[:, J * 128 : (J + 1) * 128],
                            q_ap,
                            start=True,
                            stop=True,
                        )
                        s_psums[J] = s_psum

                    def emit_exp_mask(J):
                        pt = ppool.tile([128, 512], f32r, tag="pt")
                        nc.scalar.activation(out=pt, in_=s_psums[J], func=Act.Exp)
                        j0 = J * 128
                        if j0 > i0 - 128:  # diagonal block: mask j > i
                            nc.gpsimd.affine_select(
                                out=pt,
                                in_=pt,
                                compare_op=Alu.is_ge,
                                fill=0.0,
                                base=i0 - j0,
                                pattern=[[1, 512]],
                                channel_multiplier=-1,
                            )
                        pts[J] = pt

                    def emit_attnv(J):
                        nc.tensor.matmul(
                            u_psum[0 : DH + 1, :],
                            vh[:, J, h, :],
                            pts[J],
                            start=(J == 0),
                            stop=(J == n_j - 1),
                        )

                    # software pipeline: scores run 1 tile ahead of attnV
                    emit_scores(0)
                    emit_exp_mask(0)
                    for J in range(1, n_j):
                        emit_scores(J)
                        emit_exp_mask(J)
                        emit_attnv(J - 1)
                    emit_attnv(n_j - 1)

                    recip = small.tile([1, 512], f32r, tag="recip")
                    with nc.allow_low_precision(reason="fp32r is fp32-width"):
                        nc.vector.reciprocal(recip, u_psum[DH : DH + 1, :])
                    bc_psum = psB.tile([DH, 512], f32, tag="bc")
                    nc.tensor.matmul(bc_psum, ones64, recip, start=True, stop=True)
                    bc_sb = small.tile([DH, 512], f32, tag="bcsb")
                    nc.vector.tensor_copy(bc_sb, bc_psum)
                    nc.vector.tensor_mul(
                        ct[pt_hi : pt_hi + DH, nt, i0 : i0 + 512],
                        u_psum[0:DH, :],
                        bc_sb,
                    )

                # output projection for this i-slice
                for it in range(4):
                    r0 = i0 + it * 128
                    out_sb = osb.tile([128, D], f32, tag="out")
                    for nn in range(2):
                        o_psum = ps.tile([128, 512], f32, tag="ps")
                        for t in range(2):
                            nc.tensor.matmul(
                                o_psum,
                                ct[:, t, r0 : r0 + 128],
                                wo_t[:, t, nn * 512 : (nn + 1) * 512],
                                start=(t == 0),
                                stop=(t == 1),
                            )
                        nc.vector.tensor_copy(out_sb[:, nn * 512 : (nn + 1) * 512], o_psum)
                    nc.sync.dma_start(out=outp[r0 : r0 + 128, :], in_=out_sb)

    nc.compile()
    return nc


def xq_like_w(w):
    return w[:].rearrange("(kt p) n -> p kt n", p=128)


def _get_nc():
    global _cached
    if _cached is None:
        _cached = _build()
    return _cached


def _in_maps(q, k, v, wq, bq, wk, bk, wv, bv, wo, bo):
    maps = []
    for c in range(8):
        b, g = c // G, c % G
        cs = slice(g * GD, (g + 1) * GD)
        maps.append(
            {
                "xq": np.ascontiguousarray(q[b].T).astype(np.float32, copy=False),
                "xk": np.ascontiguousarray(k[b].T).astype(np.float32, copy=False),
                "xv": np.ascontiguousarray(v[b].T).astype(np.float32, copy=False),
                "wqg": np.ascontiguousarray(wq[:, cs]),
                "wkg": np.ascontiguousarray(wk[:, cs]),
                "wvg": np.ascontiguousarray(wv[:, cs]),
                "wog": np.ascontiguousarray(wo[cs, :]),
                "bqg": np.ascontiguousarray(bq[cs]).reshape(2, 128, 1),
                "bkg": np.ascontiguousarray(bk[cs]).reshape(2, 128, 1),
            }
        )
    return maps


def run(inputs, trace=False, trace_kwargs=None):
    from concourse.bass_utils import run_bass_kernel_spmd

    nc = _get_nc()
    maps = _in_maps(**inputs)
    res = run_bass_kernel_spmd(
        nc, maps, list(range(8)), trace=trace, **(trace_kwargs or {})
    )
    q = inputs["q"]
    out = np.zeros((B, S, D), np.float32)
    for c in range(8):
        out[c // G] += res.results[c]["outp"]
    # exact bias fold: C = U/colsum + 1 (x) bv  =>  out += bv @ wo + bo
    out += inputs["bv"].astype(np.float32) @ inputs["wo"].astype(np.float32)
    out += inputs["bo"].astype(np.float32)
    return out.astype(np.float32), res


def kernel(**inputs) -> np.ndarray:
    out, _ = run(inputs)
    return out
